# revision 51
# baseline (speedup 1.0000x reference)
"""BotGCN Trainium2 kernel: 8-core SPMD Bass/Tile implementation (V3).

Model: 4 input MLPs (768/768/6/11 -> 32 each) -> concat(128) -> Linear(128) ->
GCNConv -> GCNConv -> Linear(128) -> Linear(2), LeakyReLU(0.01) activations,
symmetric-normalized graph conv with self-loops on a 100K-node/3.2M-edge graph.

Sharding: nodes split contiguously across 8 cores (12500 each); edges assigned
to the core owning their dst.

Per conv: y = dinv*(W^T x) is computed node-major per 128-node tile (PE matmul
with the feature-major resident as lhsT), AllGathered into a replicated
[8*12501, 128] f16 table (one zero pad row per shard). The scatter runs as 4
sequential passes over 25002-row table banks (dma_gather indices are int16).
Within a pass, every dst owns exactly 8 slots per bank (self edge lives in the
dst's own bank; padding slots read the bank's zero row), so a 128-slot chunk
covers 16 dsts and multiplies with a CONSTANT one-hot [128,16]. Overflow edges
(>8 per dst/bank) go through a few dynamic one-hot chunks built on DVE.
Gathers run 64 chunks per SWDGE dma_gather instruction (994ns overhead
amortized); indices stream from DRAM. PSUM accumulates per 512-dst bank with
a PE zeroing matmul; pass 0 drains via scalar-engine copy, passes 1-3 via DVE
adds into the f16 feature-major z' resident.

Dst-side dinv and conv biases are deferred algebraically: z = dinv*z' + b is
never materialized; the next matmul uses dinv^2 scales plus a rank-1
((1/dinv) x W^T b) correction accumulated into PSUM.
"""
import sys
sys.path.insert(0, "/opt/trn_rl_repo")

import numpy as np

import concourse.bacc as bacc
import concourse.bass as bass
import concourse.mybir as mybir
import concourse.tile as tile
from concourse import library_config
from concourse.bass_utils import run_bass_kernel_spmd

F16 = mybir.dt.float16
F32 = mybir.dt.float32
I16 = mybir.dt.int16

NCORES = 8
N = 100000
E = 3200000
D = 128
SLOPE = 0.01
NC = N // NCORES            # 12500 nodes per core
NST = (NC + 127) // 128     # 98 128-dst supertiles per core
NBANK = (NC + 511) // 512   # 25 PSUM groups of 512 dsts
TPS = NST + 1               # row-runs per partition in the permuted shard
NCP1 = 128 * TPS            # shard rows (12672): node i at (i%128)*TPS+i//128
GB = 8                      # fixed gather slots per (dst, table-bank)
NTB = 4                     # table banks (int16 index range)
BNK = 2 * NCP1              # rows per table bank (25344 <= 32767)
KCH = 32                    # chunks per dma_gather instruction
GBUFS = 6                   # gather buffers in flight
GI = 8                      # gather instructions per idx-tile DMA
NODE_CHUNK = 500            # free-dim chunk for feature-major matmuls
NCHUNKS = NC // NODE_CHUNK
SB = 8                      # dyn one-hot group size (chunks per DVE op)

_cache = {}
_last = None
_DEBUG = False
_ABLATE = None  # None | "noscatter" | "nocoll" | "nogather"

# fixed chunks (16 dsts each) per supertile
FCS = [(min(128, NC - st * 128) + 15) // 16 for st in range(NST)]


def _build_program(dyncpt):
    """Build the SPMD Bass program.

    dyncpt: [NTB][NST] overflow chunk count per (table bank, supertile),
    uniform across cores (max-padded).
    """
    ndyn = int(sum(sum(b) for b in dyncpt))
    ndmax = max(1, max(max(b) for b in dyncpt))
    nfix = int(sum(FCS)) * NTB
    nch = nfix + ndyn                     # total chunks per conv
    nslot = nch * 128

    nc = bacc.Bacc("TRN2", target_bir_lowering=False, debug=False,
                   num_devices=NCORES, num_swdge_queues=4)

    # des/tweet interleaved per node-chunk: [128, chunk, ktile, col] so each
    # chunk loads with one DMA of 6KB-contiguous per-partition descriptors
    desT = nc.dram_tensor("desT", [128, 6 * NC], F16, kind="ExternalInput")
    tweetT = nc.dram_tensor("tweetT", [128, 6 * NC], F16,
                            kind="ExternalInput")
    numT = nc.dram_tensor("numT", [6, NC], F16, kind="ExternalInput")
    catT = nc.dram_tensor("catT", [11, NC], F16, kind="ExternalInput")
    w_des = nc.dram_tensor("w_des", [128, 6 * 32], F16, kind="ExternalInput")
    w_tweet = nc.dram_tensor("w_tweet", [128, 6 * 32], F16, kind="ExternalInput")
    w_num = nc.dram_tensor("w_num", [6, 32], F16, kind="ExternalInput")
    w_cat = nc.dram_tensor("w_cat", [11, 32], F16, kind="ExternalInput")
    w_in = nc.dram_tensor("w_in", [128, 128], F16, kind="ExternalInput")
    w_g1 = nc.dram_tensor("w_g1", [128, 128], F16, kind="ExternalInput")
    w_g2 = nc.dram_tensor("w_g2", [128, 128], F16, kind="ExternalInput")
    w_o1 = nc.dram_tensor("w_o1", [128, 128], F16, kind="ExternalInput")
    w_o2 = nc.dram_tensor("w_o2", [128, 2], F16, kind="ExternalInput")
    b_cat4 = nc.dram_tensor("b_cat4", [128, 1], F32, kind="ExternalInput")
    b_in = nc.dram_tensor("b_in", [128, 1], F32, kind="ExternalInput")
    c2 = nc.dram_tensor("c2", [1, 128], F16, kind="ExternalInput")
    c3 = nc.dram_tensor("c3", [128, 1], F32, kind="ExternalInput")
    b_o2r = nc.dram_tensor("b_o2r", [128, 2], F32, kind="ExternalInput")
    dinv_rep = nc.dram_tensor("dinv_rep", [128, NC], F16, kind="ExternalInput")
    dinv_nm = nc.dram_tensor("dinv_nm", [128, NST], F32, kind="ExternalInput")
    dinv2_nm = nc.dram_tensor("dinv2_nm", [128, NST], F32,
                              kind="ExternalInput")
    invd1p = nc.dram_tensor("invd1p", [1, NC], F16, kind="ExternalInput")
    idx16 = nc.dram_tensor("idx16", [128, nslot // 16], I16,
                           kind="ExternalInput")
    dld = nc.dram_tensor("dld", [128, max(ndyn, 1)], F16,
                         kind="ExternalInput")
    iota128 = nc.dram_tensor("iota128", [128, 128], F16, kind="ExternalInput")
    sfix = nc.dram_tensor("sfix", [128, 16], F16, kind="ExternalInput")

    out = nc.dram_tensor("out", [NC, 2], F32, kind="ExternalOutput")
    if _DEBUG:
        dbg_y = nc.dram_tensor("dbg_y", [128, NST * 128], F16,
                               kind="ExternalOutput")
        dbg_z1 = nc.dram_tensor("dbg_z1", [128, NC], F16,
                                kind="ExternalOutput")

    with tile.TileContext(nc) as tc:
        with (
            tc.tile_pool(name="const", bufs=1) as cpool,
            tc.tile_pool(name="resid", bufs=1) as rpool,
            tc.tile_pool(name="stream", bufs=3) as spool,
            tc.tile_pool(name="pbig", bufs=2) as bpool,
            tc.tile_pool(name="psum", bufs=4, space="PSUM") as ppool,
            tc.tile_pool(name="gather", bufs=GBUFS) as gpool,
            tc.tile_pool(name="gidx", bufs=3) as ipool,
            tc.tile_pool(name="sbuild", bufs=2) as sbpool,
            tc.tile_pool(name="zpsum", bufs=3, space="PSUM") as zpool,
            tc.tile_pool(name="dram", bufs=1, space="DRAM") as dpool,
        ):
            nc.gpsimd.load_library(library_config.mlp)

            def load_const(t, shape, dt, name):
                s = cpool.tile(shape, dt, name=name)
                nc.sync.dma_start(s[:], t[:])
                return s

            s_wdes = load_const(w_des, [128, 6 * 32], F16, "s_wdes")
            s_wtweet = load_const(w_tweet, [128, 6 * 32], F16, "s_wtweet")
            s_wnum = load_const(w_num, [6, 32], F16, "s_wnum")
            s_wcat = load_const(w_cat, [11, 32], F16, "s_wcat")
            s_win = load_const(w_in, [128, 128], F16, "s_win")
            s_wg1 = load_const(w_g1, [128, 128], F16, "s_wg1")
            s_wg2 = load_const(w_g2, [128, 128], F16, "s_wg2")
            s_wo1 = load_const(w_o1, [128, 128], F16, "s_wo1")
            s_wo2 = load_const(w_o2, [128, 2], F16, "s_wo2")
            s_bcat4 = load_const(b_cat4, [128, 1], F32, "s_bcat4")
            s_bin = load_const(b_in, [128, 1], F32, "s_bin")
            s_c2 = load_const(c2, [1, 128], F16, "s_c2")
            s_c3 = load_const(c3, [128, 1], F32, "s_c3")
            s_bo2r = load_const(b_o2r, [128, 2], F32, "s_bo2r")
            s_dnm = load_const(dinv_nm, [128, NST], F32, "s_dnm")
            s_d2nm = load_const(dinv2_nm, [128, NST], F32, "s_d2nm")
            s_invd = load_const(invd1p, [1, NC], F16, "s_invd")
            s_dld = load_const(dld, [128, max(ndyn, 1)], F16, "s_dld")
            s_iota = load_const(iota128, [128, 128], F16, "s_iota")
            s_sfix = load_const(sfix, [128, 16], F16, "s_sfix")

            # big feature-major residents; disjoint lifetimes share slots
            x2T = rpool.tile([128, NC], F16, name="x2T", tag="bigA")
            ystage = rpool.tile([128, TPS * 128], F16, name="ystage",
                                tag="bigY")

            y_shards = [dpool.tile([NCP1, D], F16, name=f"y_shard{i}")
                        for i in range(2)]
            y_fulls = [dpool.tile([NCORES * NCP1, D], F16, name=f"y_full{i}",
                                  addr_space="Shared") for i in range(2)]

            lrelu = mybir.ActivationFunctionType.Lrelu
            copyf = mybir.ActivationFunctionType.Copy

            # zero pad block lives at t=NST of ystage (zeroed once); z512
            # zeroes PSUM banks via PE
            zt = cpool.tile([128, 128], F16, name="zrow")
            nc.vector.memset(zt[:], 0.0)
            z512 = cpool.tile([1, 512], F16, name="z512")
            nc.vector.memset(z512[:], 0.0)
            nc.vector.memset(ystage[:, NST * 128:], 0.0)
            # unwritten ystage rows (tail tile) must be zero, not garbage;
            # partition offset must be 32-aligned, rows 64..83 are rewritten
            # by the conv y-stage afterwards
            nc.vector.memset(ystage[64:, (NST - 1) * 128:NST * 128], 0.0)

            # ================= Phase 1: input MLPs =================
            for ci in range(NCHUNKS):
                sl = slice(ci * NODE_CHUNK, (ci + 1) * NODE_CHUNK)
                ps = ppool.tile([128, NODE_CHUNK], F32, tag="ps", name="p1ps")
                # one 6KB-descriptor DMA per chunk for each 768-dim input
                xdes = bpool.tile([128, 6 * NODE_CHUNK], F16, tag="pdes",
                                  name="pdes")
                nc.sync.dma_start(
                    xdes[:], desT[:, ci * 6 * NODE_CHUNK:
                                  (ci + 1) * 6 * NODE_CHUNK])
                xtw = bpool.tile([128, 6 * NODE_CHUNK], F16, tag="ptw",
                                 name="ptw")
                nc.scalar.dma_start(
                    xtw[:], tweetT[:, ci * 6 * NODE_CHUNK:
                                   (ci + 1) * 6 * NODE_CHUNK])
                for base, wT, big, xT, kdim in (
                    (0, s_wdes, xdes, None, 768),
                    (32, s_wtweet, xtw, None, 768),
                    (64, s_wnum, None, numT, 6),
                    (96, s_wcat, None, catT, 11),
                ):
                    nkt = (kdim + 127) // 128
                    for kt in range(nkt):
                        kw = min(128, kdim - kt * 128)
                        if big is not None:
                            wslice = wT[:, kt * 32:(kt + 1) * 32]
                            operand = big[:, kt * NODE_CHUNK:
                                          (kt + 1) * NODE_CHUNK]
                        else:
                            wslice = wT[:kw, :]
                            xt = spool.tile([128, NODE_CHUNK], F16,
                                            tag="p1x", name="p1x")
                            nc.sync.dma_start(xt[:kw, :], xT[:kw, sl])
                            operand = xt[:kw, :]
                        nc.tensor.matmul(
                            ps[base:base + 32, :],
                            wslice,
                            operand,
                            start=(kt == 0),
                            stop=(kt == nkt - 1),
                            tile_position=(0, base),
                        )
                x1t = spool.tile([128, NODE_CHUNK], F16, tag="x1t", name="x1t")
                nc.scalar.activation(x1t[:], ps[:], lrelu,
                                     bias=s_bcat4[:], alpha=SLOPE)
                ps2 = ppool.tile([128, NODE_CHUNK], F32, tag="ps", name="p2ps")
                nc.tensor.matmul(ps2[:], s_win[:], x1t[:], start=True, stop=True)
                nc.scalar.activation(x2T[:, sl], ps2[:], lrelu,
                                     bias=s_bin[:], alpha=SLOPE)

            # ================= GCN convs =================
            qn = [0]

            def conv(src_fm, wg, scale_nm, rank1, y_shard, y_full, dst_fm):
                # ---- y-stage: node-major y tiles = scale * (W^T src) ----
                for t in range(NST):
                    w = min(128, NC - t * 128)
                    ps = ppool.tile([128, 128], F32, tag="ps", name="yps")
                    nc.tensor.matmul(ps[:w, :], src_fm[:, t * 128:t * 128 + w],
                                     wg[:], start=True, stop=not rank1)
                    if rank1:
                        nc.tensor.matmul(
                            ps[:w, :], s_invd[:, t * 128:t * 128 + w],
                            s_c2[:], start=False, stop=True,
                            skip_group_check=True)
                    nc.vector.tensor_scalar(
                        out=ystage[:w, t * 128:(t + 1) * 128],
                        in0=ps[:w, :], scalar1=scale_nm[:w, t:t + 1],
                        scalar2=None, op0=mybir.AluOpType.mult)

                # permuted shard: row (i%128)*TPS + i//128 <- one linear DMA
                nc.sync.dma_start(
                    y_shard[:].rearrange("(p t) f -> p t f", p=128),
                    ystage[:].rearrange("p (t f) -> p t f", f=128))

                if _ABLATE not in ("nocoll", "base"):
                    nc.gpsimd.collective_compute(
                        "AllGather", mybir.AluOpType.bypass,
                        replica_groups=[list(range(NCORES))],
                        ins=[y_shard.opt()], outs=[y_full.opt()],
                    )

                # ---- scatter: 4 banked passes over the y table ----
                ch = 0          # global chunk cursor (matches slot stream)
                dyn0 = 0        # dyn chunk cursor
                mt = None

                istate = [None, 0]  # current idx tile, base chunk

                def fetch(ch, bank, pch0, pnch):
                    """Gather next chunks of this pass (bank-local)."""
                    n = min(KCH, pnch - (ch - pch0))
                    if istate[0] is None or (ch - istate[1] + n) > GI * KCH:
                        ni = min(GI * KCH, nch - ch)
                        it = ipool.tile([128, GI * KCH * 8], I16, tag="idx",
                                        name="idx")
                        nc.sync.dma_start(it[:, :ni * 8],
                                          idx16[:, ch * 8: ch * 8 + ni * 8])
                        istate[0] = it
                        istate[1] = ch
                    it = istate[0]
                    i0 = (ch - istate[1]) * 8
                    t_ = gpool.tile([128, KCH * 128], F16, tag="mt",
                                    name="mt")
                    nc.gpsimd.dma_gather(
                        t_[:, :n * 128].rearrange("p (c f) -> p c f", f=128),
                        y_full[bank * BNK:(bank + 1) * BNK, :],
                        it[:, i0:i0 + n * 8], n * 128, n * 128, 128,
                        queue_num=qn[0] % 4, single_packet=False)
                    qn[0] += 1
                    return t_, ch + n

                if _ABLATE in ("noscatter", "base"):
                    for g in range(NBANK):
                        bw = min(512, NC - g * 512)
                        zps = zpool.tile([128, 512], F32, tag="zps",
                                         name="zps")
                        nc.tensor.matmul(zps[:], zt[0:1, :], z512[:],
                                         start=True, stop=True,
                                         skip_group_check=True)
                        nc.scalar.activation(
                            dst_fm[:, g * 512: g * 512 + bw],
                            zps[:, :bw], copyf)
                    return

                for bank in range(NTB):
                    pch0 = ch
                    pnch = sum(FCS) + int(sum(dyncpt[bank]))
                    nxt = ch
                    for g in range(NBANK):
                        bw = min(512, NC - g * 512)
                        zps = zpool.tile([128, 512], F32, tag="zps",
                                         name="zps")
                        nc.tensor.matmul(zps[:], zt[0:1, :], z512[:],
                                         start=True, stop=False,
                                         skip_group_check=True)
                        sts = list(range(g * 4, min(g * 4 + 4, NST)))
                        nchg = sum(FCS[st] + dyncpt[bank][st] for st in sts)
                        done = 0
                        for st in sts:
                            w0 = (st % 4) * 128
                            for c in range(FCS[st]):
                                if ch == nxt:
                                    mt, nxt = fetch(ch, bank, pch0, pnch)
                                    mtb = ch
                                lhs = mt[:, (ch - mtb) * 128:
                                         (ch - mtb + 1) * 128]
                                nc.tensor.matmul(
                                    zps[:, w0 + 16 * c: w0 + 16 * c + 16],
                                    lhs, s_sfix[:],
                                    start=False, stop=(done == nchg - 1),
                                    skip_group_check=True)
                                ch += 1
                                done += 1
                            nd = dyncpt[bank][st]
                            if nd > 0:
                                stile = sbpool.tile([128, ndmax * 128], F16,
                                                    tag="stile", name="stile")
                                for g0 in range(0, nd, SB):
                                    gn = min(SB, nd - g0)
                                    dl = s_dld[:, dyn0 + g0: dyn0 + g0 + gn]
                                    nc.vector.tensor_tensor(
                                        out=stile[:, g0 * 128:(g0 + gn) * 128]
                                        .rearrange("p (c w) -> p c w", c=gn),
                                        in0=s_iota[:].unsqueeze(1)
                                        .to_broadcast([128, gn, 128]),
                                        in1=dl.unsqueeze(2)
                                        .to_broadcast([128, gn, 128]),
                                        op=mybir.AluOpType.is_equal)
                                for j in range(nd):
                                    if ch == nxt:
                                        mt, nxt = fetch(ch, bank, pch0, pnch)
                                        mtb = ch
                                    lhs = mt[:, (ch - mtb) * 128:
                                             (ch - mtb + 1) * 128]
                                    nc.tensor.matmul(
                                        zps[:, w0:w0 + 128],
                                        lhs, stile[:, j * 128:(j + 1) * 128],
                                        start=False,
                                        stop=(done == nchg - 1),
                                        skip_group_check=True)
                                    ch += 1
                                    done += 1
                                dyn0 += nd
                        # drain/accumulate this PSUM group
                        if bank == 0:
                            nc.scalar.activation(
                                dst_fm[:, g * 512: g * 512 + bw],
                                zps[:, :bw], copyf)
                        else:
                            nc.vector.tensor_tensor(
                                out=dst_fm[:, g * 512: g * 512 + bw],
                                in0=zps[:, :bw],
                                in1=dst_fm[:, g * 512: g * 512 + bw],
                                op=mybir.AluOpType.add)
                assert ch == nch and dyn0 == ndyn

            z1 = rpool.tile([128, NC], F16, name="z1", tag="bigB")
            conv(x2T, s_wg1, s_dnm, False, y_shards[0], y_fulls[0], z1)
            if _DEBUG:
                nc.sync.dma_start(dbg_y[:], ystage[:])
                nc.sync.dma_start(dbg_z1[:], z1[:])
            z2 = rpool.tile([128, NC], F16, name="z2", tag="bigA")
            conv(z1, s_wg2, s_d2nm, True, y_shards[1], y_fulls[1], z2)

            # ================= Output head =================
            # ystage is dead after conv2's AllGather: reuse its slot for the
            # replicated dinv table the head needs
            s_dinvrep = rpool.tile([128, NC], F16, name="s_dinvrep",
                                   tag="bigY")
            nc.sync.dma_start(s_dinvrep[:], dinv_rep[:])
            o1T = rpool.tile([128, NC], F16, name="o1T", tag="bigB")
            for ci in range(NCHUNKS):
                sl = slice(ci * NODE_CHUNK, (ci + 1) * NODE_CHUNK)
                ps = ppool.tile([128, NODE_CHUNK], F32, tag="ps", name="o1ps")
                nc.tensor.matmul(ps[:], s_wo1[:], z2[:, sl],
                                 start=True, stop=True)
                t1 = spool.tile([128, NODE_CHUNK], F16, tag="o1t", name="o1t")
                nc.vector.tensor_tensor(out=t1[:], in0=ps[:],
                                        in1=s_dinvrep[:, sl],
                                        op=mybir.AluOpType.mult)
                nc.scalar.activation(o1T[:, sl], t1[:], lrelu,
                                     bias=s_c3[:], alpha=SLOPE)

            ostage = rpool.tile([128, 2 * NST], F32, name="ostage",
                                tag="ostage")
            for t in range(NST):
                nlo = t * 128
                nhi = min(nlo + 128, NC)
                w = nhi - nlo
                ps = ppool.tile([128, 2], F32, tag="ps", name="o2ps")
                nc.tensor.matmul(ps[:w, :], o1T[:, nlo:nhi], s_wo2[:],
                                 start=True, stop=True)
                nc.vector.tensor_tensor(out=ostage[:w, 2 * t:2 * t + 2],
                                        in0=ps[:w, :], in1=s_bo2r[:w, :],
                                        op=mybir.AluOpType.add)
            # out[t*128 + p, c] = ostage[p, 2t + c]
            nfull = (NST - 1) * 128  # 12416 full-tile rows
            nc.sync.dma_start(
                out[:nfull, :].rearrange("(t p) c -> p t c", p=128),
                ostage[:, : 2 * (NST - 1)]
                .rearrange("p (t c) -> p t c", c=2))
            nc.sync.dma_start(out[nfull:, :],
                              ostage[: NC - nfull, 2 * (NST - 1):])

    nc.compile()
    return nc


def _prepare(edge_index):
    """Host-side graph prep: banked fixed-capacity slots + overflow chunks."""
    src = np.asarray(edge_index[0], dtype=np.int64)
    dst = np.asarray(edge_index[1], dtype=np.int64)
    deg = np.bincount(dst, minlength=N).astype(np.float64) + 1.0
    dinv = (1.0 / np.sqrt(deg)).astype(np.float32)

    order = np.argsort(dst, kind="stable")
    src_s, dst_s = src[order], dst[order]
    offs = np.searchsorted(dst_s, np.arange(0, N + 1, NC))

    cores = []
    ocnts = np.zeros((NCORES, NTB, NST), dtype=np.int64)
    for c in range(NCORES):
        s0, s1 = offs[c], offs[c + 1]
        gsrc = src_s[s0:s1]
        dl = (dst_s[s0:s1] - c * NC).astype(np.int64)
        si = gsrc % NC
        row = (gsrc // NC) * NCP1 + (si % 128) * TPS + si // 128
        bke = row // BNK
        loc = (row - bke * BNK).astype(np.int16)
        own = c // 2
        # rank within (dst, bank)
        o2 = np.lexsort((bke, dl))
        dl2, bk2, lc2 = dl[o2], bke[o2], loc[o2]
        gidkey = dl2 * NTB + bk2
        gstart = np.searchsorted(gidkey, np.arange(NC * NTB + 1))
        r2 = np.arange(dl2.size) - gstart[gidkey]
        cap = np.where(bk2 == own, GB - 1, GB)
        slot = np.where(bk2 == own, 1 + r2, r2)
        fmask = r2 < cap
        # padding entries point at spread-out zero rows (t=NST runs)
        t_ = np.arange(NC * NTB * GB) % 256
        zspread = ((t_ // 128) * NCP1 + (t_ % 128) * TPS + NST
                   ).astype(np.int16)
        fixed = zspread.reshape(NC, NTB, GB).copy()
        i_ = np.arange(NC)
        srow = (c % 2) * NCP1 + (i_ % 128) * TPS + i_ // 128
        fixed[:, own, 0] = srow.astype(np.int16)
        fixed[dl2[fmask], bk2[fmask], slot[fmask]] = lc2[fmask]
        # overflow, ordered by (bank, dst)
        olc = lc2[~fmask]
        odl = dl2[~fmask]
        obk = bk2[~fmask]
        oo = np.lexsort((odl, obk))
        olc, odl, obk = olc[oo], odl[oo], obk[oo]
        ost = odl // 128
        for b in range(NTB):
            m = obk == b
            ocnts[c, b] = np.bincount(ost[m], minlength=NST)
        cores.append((fixed, olc, odl, obk))

    dyncpt = tuple(
        tuple(int(x) for x in (ocnts[:, b, :].max(axis=0) + 127) // 128)
        for b in range(NTB))
    ndyn = int(sum(sum(b) for b in dyncpt))
    nfix = int(sum(FCS)) * NTB
    nch = nfix + ndyn
    nslot = nch * 128

    idx16 = np.zeros((NCORES, 128, nslot // 16), dtype=np.int16)
    dld = np.full((NCORES, 128, max(ndyn, 1)), -1.0, dtype=np.float16)
    for c in range(NCORES):
        fixed, olc, odl, obk = cores[c]
        locs = np.zeros(nslot, dtype=np.int16)
        pos = 0
        dyn0 = 0
        for b in range(NTB):
            m = obk == b
            blc, bdl = olc[m], odl[m]
            bst = bdl // 128
            o_starts = np.searchsorted(bst, np.arange(NST + 1))
            for st in range(NST):
                nds = min(128, NC - st * 128)
                fc = FCS[st]
                t_ = np.arange(fc * 16 * GB) % 256
                blk = ((t_ // 128) * NCP1 + (t_ % 128) * TPS + NST
                       ).astype(np.int16).reshape(fc * 16, GB)
                blk[:nds] = fixed[st * 128: st * 128 + nds, b, :]
                locs[pos:pos + fc * 128] = blk.reshape(-1)
                pos += fc * 128
                nd = dyncpt[b][st]
                if nd:
                    a, e = o_starts[st], o_starts[st + 1]
                    cnt = e - a
                    # dyn padding: spread data rows; their one-hot row is 0
                    buf = ((np.arange(nd * 128) * 131) % NCP1
                           ).astype(np.int16)
                    buf[:cnt] = blc[a:e]
                    lbuf = np.full(nd * 128, -1.0, dtype=np.float16)
                    lbuf[:cnt] = (bdl[a:e] - st * 128).astype(np.float16)
                    locs[pos:pos + nd * 128] = buf
                    dld[c, :, dyn0:dyn0 + nd] = lbuf.reshape(nd, 128).T
                    pos += nd * 128
                    dyn0 += nd
        assert pos == nslot and dyn0 == ndyn
        # wrapped int16 layout: index j lives at [j%16, j//16]
        iw = locs.reshape(-1, 16).T
        idx16[c] = np.tile(iw, (8, 1))
    return dinv, dyncpt, idx16, dld


def kernel(des, tweet, num_prop, cat_prop, edge_index, edge_type,
           W_des, b_des, W_tweet, b_tweet, W_num, b_num, W_cat, b_cat,
           W_in, b_in, W_g1, b_g1, W_g2, b_g2, W_o1, b_o1, W_o2, b_o2):
    des = np.asarray(des, dtype=np.float32)
    tweet = np.asarray(tweet, dtype=np.float32)
    num_prop = np.asarray(num_prop, dtype=np.float32)
    cat_prop = np.asarray(cat_prop, dtype=np.float32)
    edge_index = np.asarray(edge_index)

    dinv, dyncpt, idx16, dld = _prepare(edge_index)

    key = ("prog", dyncpt, _DEBUG, _ABLATE, KCH, GBUFS)
    if key not in _cache:
        _cache[key] = _build_program(dyncpt)
    nc = _cache[key]

    f16 = np.float16
    cat4_bias = np.concatenate(
        [np.asarray(b) for b in (b_des, b_tweet, b_num, b_cat)]
    ).astype(np.float32)
    iota128 = np.tile(np.arange(128, dtype=np.float16)[None, :], (128, 1))
    sfix = np.zeros((128, 16), dtype=np.float16)
    sfix[np.arange(128), np.arange(128) // GB] = 1.0
    b_o2r = np.tile(np.asarray(b_o2, dtype=np.float32)[None, :], (128, 1))
    c2 = (np.asarray(b_g1, np.float64) @ np.asarray(W_g2, np.float64)
          ).astype(f16)[None, :]
    c3 = (np.asarray(b_g2, np.float64) @ np.asarray(W_o1, np.float64)
          + np.asarray(b_o1, np.float64)).astype(np.float32)[:, None]

    in_maps = []
    for c in range(NCORES):
        sl = slice(c * NC, (c + 1) * NC)
        dv = dinv[sl]
        dnm = np.zeros((128, NST), dtype=np.float32)
        dnm.T.flat[:NC] = dv
        d2nm = np.zeros((128, NST), dtype=np.float32)
        d2nm.T.flat[:NC] = dv * dv
        def inter(a):
            # [NC, 768] -> [128, chunk, ktile, col] flattened
            t = a.T.astype(f16).reshape(6, 128, NCHUNKS, NODE_CHUNK)
            return np.ascontiguousarray(
                t.transpose(1, 2, 0, 3).reshape(128, 6 * NC))
        m = {
            "desT": inter(des[sl]),
            "tweetT": inter(tweet[sl]),
            "numT": np.ascontiguousarray(num_prop[sl].T).astype(f16),
            "catT": np.ascontiguousarray(cat_prop[sl].T).astype(f16),
            "w_des": np.ascontiguousarray(
                np.asarray(W_des, f16).reshape(6, 128, 32)
                .transpose(1, 0, 2).reshape(128, 192)),
            "w_tweet": np.ascontiguousarray(
                np.asarray(W_tweet, f16).reshape(6, 128, 32)
                .transpose(1, 0, 2).reshape(128, 192)),
            "w_num": np.asarray(W_num, f16), "w_cat": np.asarray(W_cat, f16),
            "w_in": np.asarray(W_in, f16), "w_g1": np.asarray(W_g1, f16),
            "w_g2": np.asarray(W_g2, f16), "w_o1": np.asarray(W_o1, f16),
            "w_o2": np.asarray(W_o2, f16),
            "b_cat4": cat4_bias[:, None],
            "b_in": np.asarray(b_in, np.float32)[:, None],
            "c2": c2, "c3": c3,
            "b_o2r": b_o2r,
            "dinv_rep": np.tile(dv.astype(f16)[None, :], (128, 1)),
            "dinv_nm": dnm, "dinv2_nm": d2nm,
            "invd1p": (1.0 / dv).astype(f16)[None, :],
            "idx16": idx16[c],
            "dld": dld[c],
            "iota128": iota128,
            "sfix": sfix,
        }
        in_maps.append(m)

    global _last
    _last = (nc, in_maps)
    res = run_bass_kernel_spmd(nc, in_maps, core_ids=list(range(NCORES)))
    out = np.concatenate([res.results[c]["out"] for c in range(NCORES)],
                         axis=0)
    return out.astype(np.float32)


def prepare_run(**inputs):
    """Build (or fetch cached) program + per-core input maps, for benchmarking."""
    global _last
    kernel(**inputs)
    return _last


# revision 57
# speedup vs baseline: 1.0485x; 1.0485x over previous
"""BotGCN Trainium2 kernel: 8-core SPMD Bass/Tile implementation (V3).

Model: 4 input MLPs (768/768/6/11 -> 32 each) -> concat(128) -> Linear(128) ->
GCNConv -> GCNConv -> Linear(128) -> Linear(2), LeakyReLU(0.01) activations,
symmetric-normalized graph conv with self-loops on a 100K-node/3.2M-edge graph.

Sharding: nodes split contiguously across 8 cores (12500 each); edges assigned
to the core owning their dst.

Per conv: y = dinv*(W^T x) is computed node-major per 128-node tile (PE matmul
with the feature-major resident as lhsT), AllGathered into a replicated
[8*12501, 128] f16 table (one zero pad row per shard). The scatter runs as 4
sequential passes over 25002-row table banks (dma_gather indices are int16).
Within a pass, every dst owns exactly 8 slots per bank (self edge lives in the
dst's own bank; padding slots read the bank's zero row), so a 128-slot chunk
covers 16 dsts and multiplies with a CONSTANT one-hot [128,16]. Overflow edges
(>8 per dst/bank) go through a few dynamic one-hot chunks built on DVE.
Gathers run 64 chunks per SWDGE dma_gather instruction (994ns overhead
amortized); indices stream from DRAM. PSUM accumulates per 512-dst bank with
a PE zeroing matmul; pass 0 drains via scalar-engine copy, passes 1-3 via DVE
adds into the f16 feature-major z' resident.

Dst-side dinv and conv biases are deferred algebraically: z = dinv*z' + b is
never materialized; the next matmul uses dinv^2 scales plus a rank-1
((1/dinv) x W^T b) correction accumulated into PSUM.
"""
import sys
sys.path.insert(0, "/opt/trn_rl_repo")

import numpy as np

import concourse.bacc as bacc
import concourse.bass as bass
import concourse.mybir as mybir
import concourse.tile as tile
from concourse import library_config
from concourse.bass_utils import run_bass_kernel_spmd

F16 = mybir.dt.float16
F32 = mybir.dt.float32
I16 = mybir.dt.int16

NCORES = 8
N = 100000
E = 3200000
D = 128
SLOPE = 0.01
NC = N // NCORES            # 12500 nodes per core
NST = (NC + 127) // 128     # 98 128-dst supertiles per core
NBANK = (NC + 511) // 512   # 25 PSUM groups of 512 dsts
TPS = NST + 1               # row-runs per partition in the permuted shard
NCP1 = 128 * TPS            # shard rows (12672): node i at (i%128)*TPS+i//128
GB = 8                      # fixed gather slots per (dst, table-bank)
NTB = 4                     # table banks (int16 index range)
BNK = 2 * NCP1              # rows per table bank (25344 <= 32767)
KCH = 32                    # chunks per dma_gather instruction
GBUFS = 6                   # gather buffers in flight
GI = 8                      # gather instructions per idx-tile DMA
NODE_CHUNK = 500            # free-dim chunk for feature-major matmuls
NCHUNKS = NC // NODE_CHUNK
SB = 8                      # dyn one-hot group size (chunks per DVE op)

_cache = {}
_last = None
_DEBUG = False
_ABLATE = None  # None | "noscatter" | "nocoll" | "nogather"

# fixed chunks (16 dsts each) per supertile
FCS = [(min(128, NC - st * 128) + 15) // 16 for st in range(NST)]


def _build_program(dyncpt):
    """Build the SPMD Bass program.

    dyncpt: [NTB][NST] overflow chunk count per (table bank, supertile),
    uniform across cores (max-padded).
    """
    ndyn = int(sum(sum(b) for b in dyncpt))
    ndmax = max(1, max(max(b) for b in dyncpt))
    nfix = int(sum(FCS)) * NTB
    nch = nfix + ndyn                     # total chunks per conv
    nslot = nch * 128

    nc = bacc.Bacc("TRN2", target_bir_lowering=False, debug=False,
                   num_devices=NCORES, num_swdge_queues=4)

    # des/tweet interleaved per node-chunk: [128, chunk, ktile, col] so each
    # chunk loads with one DMA of 6KB-contiguous per-partition descriptors
    desT = nc.dram_tensor("desT", [128, 6 * NC], F16, kind="ExternalInput")
    tweetT = nc.dram_tensor("tweetT", [128, 6 * NC], F16,
                            kind="ExternalInput")
    numT = nc.dram_tensor("numT", [6, NC], F16, kind="ExternalInput")
    catT = nc.dram_tensor("catT", [11, NC], F16, kind="ExternalInput")
    w_des = nc.dram_tensor("w_des", [128, 6 * 32], F16, kind="ExternalInput")
    w_tweet = nc.dram_tensor("w_tweet", [128, 6 * 32], F16, kind="ExternalInput")
    w_num = nc.dram_tensor("w_num", [6, 32], F16, kind="ExternalInput")
    w_cat = nc.dram_tensor("w_cat", [11, 32], F16, kind="ExternalInput")
    w_in = nc.dram_tensor("w_in", [128, 128], F16, kind="ExternalInput")
    w_g1 = nc.dram_tensor("w_g1", [128, 128], F16, kind="ExternalInput")
    w_g2 = nc.dram_tensor("w_g2", [128, 128], F16, kind="ExternalInput")
    w_o1 = nc.dram_tensor("w_o1", [128, 128], F16, kind="ExternalInput")
    w_o2 = nc.dram_tensor("w_o2", [128, 2], F16, kind="ExternalInput")
    b_cat4 = nc.dram_tensor("b_cat4", [128, 1], F32, kind="ExternalInput")
    b_in = nc.dram_tensor("b_in", [128, 1], F32, kind="ExternalInput")
    c2 = nc.dram_tensor("c2", [1, 128], F16, kind="ExternalInput")
    c3 = nc.dram_tensor("c3", [128, 1], F32, kind="ExternalInput")
    b_o2r = nc.dram_tensor("b_o2r", [128, 2], F32, kind="ExternalInput")
    dinv_rep = nc.dram_tensor("dinv_rep", [128, NC], F16, kind="ExternalInput")
    dinv_nm = nc.dram_tensor("dinv_nm", [128, NST], F32, kind="ExternalInput")
    dinv2_nm = nc.dram_tensor("dinv2_nm", [128, NST], F32,
                              kind="ExternalInput")
    invd1p = nc.dram_tensor("invd1p", [1, NC], F16, kind="ExternalInput")
    idx16 = nc.dram_tensor("idx16", [128, nslot // 16], I16,
                           kind="ExternalInput")
    dld = nc.dram_tensor("dld", [128, max(ndyn, 1)], F16,
                         kind="ExternalInput")
    iota128 = nc.dram_tensor("iota128", [128, 128], F16, kind="ExternalInput")
    sfix = nc.dram_tensor("sfix", [128, 16], F16, kind="ExternalInput")
    ident = nc.dram_tensor("ident", [128, 128], F16, kind="ExternalInput")

    out = nc.dram_tensor("out", [NC, 2], F32, kind="ExternalOutput")
    if _DEBUG:
        dbg_y = nc.dram_tensor("dbg_y", [128, NST * 128], F16,
                               kind="ExternalOutput")
        dbg_z1 = nc.dram_tensor("dbg_z1", [128, NC], F16,
                                kind="ExternalOutput")

    with tile.TileContext(nc) as tc:
        with (
            tc.tile_pool(name="const", bufs=1) as cpool,
            tc.tile_pool(name="resid", bufs=1) as rpool,
            tc.tile_pool(name="stream", bufs=3) as spool,
            tc.tile_pool(name="pbig", bufs=2) as bpool,
            tc.tile_pool(name="psum", bufs=4, space="PSUM") as ppool,
            tc.tile_pool(name="gather", bufs=GBUFS) as gpool,
            tc.tile_pool(name="gidx", bufs=3) as ipool,
            tc.tile_pool(name="sbuild", bufs=2) as sbpool,
            tc.tile_pool(name="zpsum", bufs=3, space="PSUM") as zpool,
            tc.tile_pool(name="dram", bufs=1, space="DRAM") as dpool,
        ):
            nc.gpsimd.load_library(library_config.mlp)

            def load_const(t, shape, dt, name):
                s = cpool.tile(shape, dt, name=name)
                nc.sync.dma_start(s[:], t[:])
                return s

            s_wdes = load_const(w_des, [128, 6 * 32], F16, "s_wdes")
            s_wtweet = load_const(w_tweet, [128, 6 * 32], F16, "s_wtweet")
            s_wnum = load_const(w_num, [6, 32], F16, "s_wnum")
            s_wcat = load_const(w_cat, [11, 32], F16, "s_wcat")
            s_win = load_const(w_in, [128, 128], F16, "s_win")
            s_wg1 = load_const(w_g1, [128, 128], F16, "s_wg1")
            s_wg2 = load_const(w_g2, [128, 128], F16, "s_wg2")
            s_wo1 = load_const(w_o1, [128, 128], F16, "s_wo1")
            s_wo2 = load_const(w_o2, [128, 2], F16, "s_wo2")
            s_bcat4 = load_const(b_cat4, [128, 1], F32, "s_bcat4")
            s_bin = load_const(b_in, [128, 1], F32, "s_bin")
            s_c2 = load_const(c2, [1, 128], F16, "s_c2")
            s_c3 = load_const(c3, [128, 1], F32, "s_c3")
            s_bo2r = load_const(b_o2r, [128, 2], F32, "s_bo2r")
            s_dnm = load_const(dinv_nm, [128, NST], F32, "s_dnm")
            s_d2nm = load_const(dinv2_nm, [128, NST], F32, "s_d2nm")
            s_invd = load_const(invd1p, [1, NC], F16, "s_invd")
            s_dld = load_const(dld, [128, max(ndyn, 1)], F16, "s_dld")
            s_iota = load_const(iota128, [128, 128], F16, "s_iota")
            s_sfix = load_const(sfix, [128, 16], F16, "s_sfix")
            s_ident = load_const(ident, [128, 128], F16, "s_ident")

            # big feature-major residents; disjoint lifetimes share slots
            x2T = rpool.tile([128, NC], F16, name="x2T", tag="bigA")
            ystage = rpool.tile([128, TPS * 128], F16, name="ystage",
                                tag="bigY")

            y_shards = [dpool.tile([NCP1, D], F16, name=f"y_shard{i}")
                        for i in range(2)]
            y_fulls = [dpool.tile([NCORES * NCP1, D], F16, name=f"y_full{i}",
                                  addr_space="Shared") for i in range(2)]

            lrelu = mybir.ActivationFunctionType.Lrelu
            copyf = mybir.ActivationFunctionType.Copy

            # zero pad block lives at t=NST of ystage (zeroed once); z512
            # zeroes PSUM banks via PE
            zt = cpool.tile([128, 128], F16, name="zrow")
            nc.vector.memset(zt[:], 0.0)
            z512 = cpool.tile([1, 512], F16, name="z512")
            nc.vector.memset(z512[:], 0.0)
            nc.vector.memset(ystage[:, NST * 128:], 0.0)
            # unwritten ystage rows (tail tile) must be zero, not garbage;
            # partition offset must be 32-aligned, rows 64..83 are rewritten
            # by the conv y-stage afterwards
            nc.vector.memset(ystage[64:, (NST - 1) * 128:NST * 128], 0.0)

            # ================= Phase 1: input MLPs =================
            for ci in range(NCHUNKS):
                sl = slice(ci * NODE_CHUNK, (ci + 1) * NODE_CHUNK)
                ps = ppool.tile([128, NODE_CHUNK], F32, tag="ps", name="p1ps")
                # one 6KB-descriptor DMA per chunk for each 768-dim input
                xdes = bpool.tile([128, 6 * NODE_CHUNK], F16, tag="pdes",
                                  name="pdes")
                nc.sync.dma_start(
                    xdes[:], desT[:, ci * 6 * NODE_CHUNK:
                                  (ci + 1) * 6 * NODE_CHUNK])
                xtw = bpool.tile([128, 6 * NODE_CHUNK], F16, tag="ptw",
                                 name="ptw")
                nc.scalar.dma_start(
                    xtw[:], tweetT[:, ci * 6 * NODE_CHUNK:
                                   (ci + 1) * 6 * NODE_CHUNK])
                for base, wT, big, xT, kdim in (
                    (0, s_wdes, xdes, None, 768),
                    (32, s_wtweet, xtw, None, 768),
                    (64, s_wnum, None, numT, 6),
                    (96, s_wcat, None, catT, 11),
                ):
                    nkt = (kdim + 127) // 128
                    for kt in range(nkt):
                        kw = min(128, kdim - kt * 128)
                        if big is not None:
                            wslice = wT[:, kt * 32:(kt + 1) * 32]
                            operand = big[:, kt * NODE_CHUNK:
                                          (kt + 1) * NODE_CHUNK]
                        else:
                            wslice = wT[:kw, :]
                            xt = spool.tile([128, NODE_CHUNK], F16,
                                            tag="p1x", name="p1x")
                            nc.sync.dma_start(xt[:kw, :], xT[:kw, sl])
                            operand = xt[:kw, :]
                        nc.tensor.matmul(
                            ps[base:base + 32, :],
                            wslice,
                            operand,
                            start=(kt == 0),
                            stop=(kt == nkt - 1),
                            tile_position=(0, base),
                        )
                x1t = spool.tile([128, NODE_CHUNK], F16, tag="x1t", name="x1t")
                nc.scalar.activation(x1t[:], ps[:], lrelu,
                                     bias=s_bcat4[:], alpha=SLOPE)
                ps2 = ppool.tile([128, NODE_CHUNK], F32, tag="ps", name="p2ps")
                nc.tensor.matmul(ps2[:], s_win[:], x1t[:], start=True, stop=True)
                nc.scalar.activation(x2T[:, sl], ps2[:], lrelu,
                                     bias=s_bin[:], alpha=SLOPE)

            # ================= GCN convs =================
            qn = [0]

            def conv(src_fm, wg, scale_nm, rank1, y_shard, y_full, dst_fm):
                # ---- y-stage: node-major y tiles = scale * (W^T src) ----
                for t in range(NST):
                    w = min(128, NC - t * 128)
                    ps = ppool.tile([128, 128], F32, tag="ps", name="yps")
                    nc.tensor.matmul(ps[:w, :], src_fm[:, t * 128:t * 128 + w],
                                     wg[:], start=True, stop=not rank1)
                    if rank1:
                        nc.tensor.matmul(
                            ps[:w, :], s_invd[:, t * 128:t * 128 + w],
                            s_c2[:], start=False, stop=True,
                            skip_group_check=True)
                    nc.vector.tensor_scalar(
                        out=ystage[:w, t * 128:(t + 1) * 128],
                        in0=ps[:w, :], scalar1=scale_nm[:w, t:t + 1],
                        scalar2=None, op0=mybir.AluOpType.mult)

                # permuted shard: row (i%128)*TPS + i//128 <- one linear DMA
                nc.sync.dma_start(
                    y_shard[:].rearrange("(p t) f -> p t f", p=128),
                    ystage[:].rearrange("p (t f) -> p t f", f=128))

                if _ABLATE not in ("nocoll", "base"):
                    nc.gpsimd.collective_compute(
                        "AllGather", mybir.AluOpType.bypass,
                        replica_groups=[list(range(NCORES))],
                        ins=[y_shard.opt()], outs=[y_full.opt()],
                    )

                # ---- scatter: 4 banked passes over the y table ----
                ch = 0          # global chunk cursor (matches slot stream)
                dyn0 = 0        # dyn chunk cursor
                mt = None

                istate = [None, 0]  # current idx tile, base chunk

                def fetch(ch, bank, pch0, pnch):
                    """Gather next chunks of this pass (bank-local)."""
                    n = min(KCH, pnch - (ch - pch0))
                    if istate[0] is None or (ch - istate[1] + n) > GI * KCH:
                        ni = min(GI * KCH, nch - ch)
                        it = ipool.tile([128, GI * KCH * 8], I16, tag="idx",
                                        name="idx")
                        nc.sync.dma_start(it[:, :ni * 8],
                                          idx16[:, ch * 8: ch * 8 + ni * 8])
                        istate[0] = it
                        istate[1] = ch
                    it = istate[0]
                    i0 = (ch - istate[1]) * 8
                    t_ = gpool.tile([128, KCH * 128], F16, tag="mt",
                                    name="mt")
                    nc.gpsimd.dma_gather(
                        t_[:, :n * 128].rearrange("p (c f) -> p c f", f=128),
                        y_full[bank * BNK:(bank + 1) * BNK, :],
                        it[:, i0:i0 + n * 8], n * 128, n * 128, 128,
                        queue_num=qn[0] % 4, single_packet=False)
                    qn[0] += 1
                    return t_, ch + n

                if _ABLATE in ("noscatter", "base"):
                    for g in range(NBANK):
                        bw = min(512, NC - g * 512)
                        zps = zpool.tile([128, 512], F32, tag="zps",
                                         name="zps")
                        nc.tensor.matmul(zps[:], zt[0:1, :], z512[:],
                                         start=True, stop=True,
                                         skip_group_check=True)
                        nc.scalar.activation(
                            dst_fm[:, g * 512: g * 512 + bw],
                            zps[:, :bw], copyf)
                    return

                for bank in range(NTB):
                    pch0 = ch
                    pnch = sum(FCS) + int(sum(dyncpt[bank]))
                    nxt = ch
                    for g in range(NBANK):
                        bw = min(512, NC - g * 512)
                        zps = zpool.tile([128, 512], F32, tag="zps",
                                         name="zps")
                        nc.tensor.matmul(zps[:], zt[0:1, :], z512[:],
                                         start=True, stop=False,
                                         skip_group_check=True)
                        sts = list(range(g * 4, min(g * 4 + 4, NST)))
                        nchg = sum(FCS[st] + dyncpt[bank][st] for st in sts)
                        done = 0
                        if bank == 0:
                            # self-loop term: transpose node-major y tiles
                            # straight from SBUF into the accumulator
                            for st in sts:
                                ws = min(128, NC - st * 128)
                                nc.tensor.matmul(
                                    zps[:, (st % 4) * 128:
                                        (st % 4) * 128 + ws],
                                    ystage[:ws, st * 128:(st + 1) * 128],
                                    s_ident[:ws, :ws],
                                    start=False, stop=False,
                                    skip_group_check=True)
                        for st in sts:
                            w0 = (st % 4) * 128
                            for c in range(FCS[st]):
                                if ch == nxt:
                                    mt, nxt = fetch(ch, bank, pch0, pnch)
                                    mtb = ch
                                lhs = mt[:, (ch - mtb) * 128:
                                         (ch - mtb + 1) * 128]
                                nc.tensor.matmul(
                                    zps[:, w0 + 16 * c: w0 + 16 * c + 16],
                                    lhs, s_sfix[:],
                                    start=False, stop=(done == nchg - 1),
                                    skip_group_check=True)
                                ch += 1
                                done += 1
                            nd = dyncpt[bank][st]
                            if nd > 0:
                                stile = sbpool.tile([128, ndmax * 128], F16,
                                                    tag="stile", name="stile")
                                for g0 in range(0, nd, SB):
                                    gn = min(SB, nd - g0)
                                    dl = s_dld[:, dyn0 + g0: dyn0 + g0 + gn]
                                    nc.vector.tensor_tensor(
                                        out=stile[:, g0 * 128:(g0 + gn) * 128]
                                        .rearrange("p (c w) -> p c w", c=gn),
                                        in0=s_iota[:].unsqueeze(1)
                                        .to_broadcast([128, gn, 128]),
                                        in1=dl.unsqueeze(2)
                                        .to_broadcast([128, gn, 128]),
                                        op=mybir.AluOpType.is_equal)
                                for j in range(nd):
                                    if ch == nxt:
                                        mt, nxt = fetch(ch, bank, pch0, pnch)
                                        mtb = ch
                                    lhs = mt[:, (ch - mtb) * 128:
                                             (ch - mtb + 1) * 128]
                                    nc.tensor.matmul(
                                        zps[:, w0:w0 + 128],
                                        lhs, stile[:, j * 128:(j + 1) * 128],
                                        start=False,
                                        stop=(done == nchg - 1),
                                        skip_group_check=True)
                                    ch += 1
                                    done += 1
                                dyn0 += nd
                        # drain/accumulate this PSUM group
                        if bank == 0:
                            nc.scalar.activation(
                                dst_fm[:, g * 512: g * 512 + bw],
                                zps[:, :bw], copyf)
                        else:
                            nc.vector.tensor_tensor(
                                out=dst_fm[:, g * 512: g * 512 + bw],
                                in0=zps[:, :bw],
                                in1=dst_fm[:, g * 512: g * 512 + bw],
                                op=mybir.AluOpType.add)
                assert ch == nch and dyn0 == ndyn

            z1 = rpool.tile([128, NC], F16, name="z1", tag="bigB")
            conv(x2T, s_wg1, s_dnm, False, y_shards[0], y_fulls[0], z1)
            if _DEBUG:
                nc.sync.dma_start(dbg_y[:], ystage[:])
                nc.sync.dma_start(dbg_z1[:], z1[:])
            z2 = rpool.tile([128, NC], F16, name="z2", tag="bigA")
            conv(z1, s_wg2, s_d2nm, True, y_shards[1], y_fulls[1], z2)

            # ================= Output head =================
            # ystage is dead after conv2's AllGather: reuse its slot for the
            # replicated dinv table the head needs
            s_dinvrep = rpool.tile([128, NC], F16, name="s_dinvrep",
                                   tag="bigY")
            nc.sync.dma_start(s_dinvrep[:], dinv_rep[:])
            o1T = rpool.tile([128, NC], F16, name="o1T", tag="bigB")
            for ci in range(NCHUNKS):
                sl = slice(ci * NODE_CHUNK, (ci + 1) * NODE_CHUNK)
                ps = ppool.tile([128, NODE_CHUNK], F32, tag="ps", name="o1ps")
                nc.tensor.matmul(ps[:], s_wo1[:], z2[:, sl],
                                 start=True, stop=True)
                t1 = spool.tile([128, NODE_CHUNK], F16, tag="o1t", name="o1t")
                nc.vector.tensor_tensor(out=t1[:], in0=ps[:],
                                        in1=s_dinvrep[:, sl],
                                        op=mybir.AluOpType.mult)
                nc.scalar.activation(o1T[:, sl], t1[:], lrelu,
                                     bias=s_c3[:], alpha=SLOPE)

            ostage = rpool.tile([128, 2 * NST], F32, name="ostage",
                                tag="ostage")
            for t in range(NST):
                nlo = t * 128
                nhi = min(nlo + 128, NC)
                w = nhi - nlo
                ps = ppool.tile([128, 2], F32, tag="ps", name="o2ps")
                nc.tensor.matmul(ps[:w, :], o1T[:, nlo:nhi], s_wo2[:],
                                 start=True, stop=True)
                nc.vector.tensor_tensor(out=ostage[:w, 2 * t:2 * t + 2],
                                        in0=ps[:w, :], in1=s_bo2r[:w, :],
                                        op=mybir.AluOpType.add)
            # out[t*128 + p, c] = ostage[p, 2t + c]
            nfull = (NST - 1) * 128  # 12416 full-tile rows
            nc.sync.dma_start(
                out[:nfull, :].rearrange("(t p) c -> p t c", p=128),
                ostage[:, : 2 * (NST - 1)]
                .rearrange("p (t c) -> p t c", c=2))
            nc.sync.dma_start(out[nfull:, :],
                              ostage[: NC - nfull, 2 * (NST - 1):])

    nc.compile()
    return nc


def _prepare(edge_index):
    """Host-side graph prep: banked fixed-capacity slots + overflow chunks."""
    src = np.asarray(edge_index[0], dtype=np.int64)
    dst = np.asarray(edge_index[1], dtype=np.int64)
    deg = np.bincount(dst, minlength=N).astype(np.float64) + 1.0
    dinv = (1.0 / np.sqrt(deg)).astype(np.float32)

    order = np.argsort(dst, kind="stable")
    src_s, dst_s = src[order], dst[order]
    offs = np.searchsorted(dst_s, np.arange(0, N + 1, NC))

    cores = []
    ocnts = np.zeros((NCORES, NTB, NST), dtype=np.int64)
    for c in range(NCORES):
        s0, s1 = offs[c], offs[c + 1]
        gsrc = src_s[s0:s1]
        dl = (dst_s[s0:s1] - c * NC).astype(np.int64)
        si = gsrc % NC
        row = (gsrc // NC) * NCP1 + (si % 128) * TPS + si // 128
        bke = row // BNK
        loc = (row - bke * BNK).astype(np.int16)
        own = c // 2
        # rank within (dst, bank)
        o2 = np.lexsort((bke, dl))
        dl2, bk2, lc2 = dl[o2], bke[o2], loc[o2]
        gidkey = dl2 * NTB + bk2
        gstart = np.searchsorted(gidkey, np.arange(NC * NTB + 1))
        r2 = np.arange(dl2.size) - gstart[gidkey]
        fmask = r2 < GB
        # padding entries point at spread-out zero rows (t=NST runs)
        t_ = np.arange(NC * NTB * GB) % 256
        zspread = ((t_ // 128) * NCP1 + (t_ % 128) * TPS + NST
                   ).astype(np.int16)
        fixed = zspread.reshape(NC, NTB, GB).copy()
        fixed[dl2[fmask], bk2[fmask], r2[fmask]] = lc2[fmask]
        # overflow, ordered by (bank, dst)
        olc = lc2[~fmask]
        odl = dl2[~fmask]
        obk = bk2[~fmask]
        oo = np.lexsort((odl, obk))
        olc, odl, obk = olc[oo], odl[oo], obk[oo]
        ost = odl // 128
        for b in range(NTB):
            m = obk == b
            ocnts[c, b] = np.bincount(ost[m], minlength=NST)
        cores.append((fixed, olc, odl, obk))

    dyncpt = tuple(
        tuple(int(x) for x in (ocnts[:, b, :].max(axis=0) + 127) // 128)
        for b in range(NTB))
    ndyn = int(sum(sum(b) for b in dyncpt))
    nfix = int(sum(FCS)) * NTB
    nch = nfix + ndyn
    nslot = nch * 128

    idx16 = np.zeros((NCORES, 128, nslot // 16), dtype=np.int16)
    dld = np.full((NCORES, 128, max(ndyn, 1)), -1.0, dtype=np.float16)
    for c in range(NCORES):
        fixed, olc, odl, obk = cores[c]
        locs = np.zeros(nslot, dtype=np.int16)
        pos = 0
        dyn0 = 0
        for b in range(NTB):
            m = obk == b
            blc, bdl = olc[m], odl[m]
            bst = bdl // 128
            o_starts = np.searchsorted(bst, np.arange(NST + 1))
            for st in range(NST):
                nds = min(128, NC - st * 128)
                fc = FCS[st]
                t_ = np.arange(fc * 16 * GB) % 256
                blk = ((t_ // 128) * NCP1 + (t_ % 128) * TPS + NST
                       ).astype(np.int16).reshape(fc * 16, GB)
                blk[:nds] = fixed[st * 128: st * 128 + nds, b, :]
                locs[pos:pos + fc * 128] = blk.reshape(-1)
                pos += fc * 128
                nd = dyncpt[b][st]
                if nd:
                    a, e = o_starts[st], o_starts[st + 1]
                    cnt = e - a
                    # dyn padding: spread data rows; their one-hot row is 0
                    buf = ((np.arange(nd * 128) * 131) % NCP1
                           ).astype(np.int16)
                    buf[:cnt] = blc[a:e]
                    lbuf = np.full(nd * 128, -1.0, dtype=np.float16)
                    lbuf[:cnt] = (bdl[a:e] - st * 128).astype(np.float16)
                    locs[pos:pos + nd * 128] = buf
                    dld[c, :, dyn0:dyn0 + nd] = lbuf.reshape(nd, 128).T
                    pos += nd * 128
                    dyn0 += nd
        assert pos == nslot and dyn0 == ndyn
        # wrapped int16 layout: index j lives at [j%16, j//16]
        iw = locs.reshape(-1, 16).T
        idx16[c] = np.tile(iw, (8, 1))
    return dinv, dyncpt, idx16, dld


def kernel(des, tweet, num_prop, cat_prop, edge_index, edge_type,
           W_des, b_des, W_tweet, b_tweet, W_num, b_num, W_cat, b_cat,
           W_in, b_in, W_g1, b_g1, W_g2, b_g2, W_o1, b_o1, W_o2, b_o2):
    des = np.asarray(des, dtype=np.float32)
    tweet = np.asarray(tweet, dtype=np.float32)
    num_prop = np.asarray(num_prop, dtype=np.float32)
    cat_prop = np.asarray(cat_prop, dtype=np.float32)
    edge_index = np.asarray(edge_index)

    dinv, dyncpt, idx16, dld = _prepare(edge_index)

    key = ("prog", dyncpt, _DEBUG, _ABLATE, KCH, GBUFS)
    if key not in _cache:
        _cache[key] = _build_program(dyncpt)
    nc = _cache[key]

    f16 = np.float16
    cat4_bias = np.concatenate(
        [np.asarray(b) for b in (b_des, b_tweet, b_num, b_cat)]
    ).astype(np.float32)
    iota128 = np.tile(np.arange(128, dtype=np.float16)[None, :], (128, 1))
    sfix = np.zeros((128, 16), dtype=np.float16)
    sfix[np.arange(128), np.arange(128) // GB] = 1.0
    b_o2r = np.tile(np.asarray(b_o2, dtype=np.float32)[None, :], (128, 1))
    c2 = (np.asarray(b_g1, np.float64) @ np.asarray(W_g2, np.float64)
          ).astype(f16)[None, :]
    c3 = (np.asarray(b_g2, np.float64) @ np.asarray(W_o1, np.float64)
          + np.asarray(b_o1, np.float64)).astype(np.float32)[:, None]

    in_maps = []
    for c in range(NCORES):
        sl = slice(c * NC, (c + 1) * NC)
        dv = dinv[sl]
        dnm = np.zeros((128, NST), dtype=np.float32)
        dnm.T.flat[:NC] = dv
        d2nm = np.zeros((128, NST), dtype=np.float32)
        d2nm.T.flat[:NC] = dv * dv
        def inter(a):
            # [NC, 768] -> [128, chunk, ktile, col] flattened
            t = a.T.astype(f16).reshape(6, 128, NCHUNKS, NODE_CHUNK)
            return np.ascontiguousarray(
                t.transpose(1, 2, 0, 3).reshape(128, 6 * NC))
        m = {
            "desT": inter(des[sl]),
            "tweetT": inter(tweet[sl]),
            "numT": np.ascontiguousarray(num_prop[sl].T).astype(f16),
            "catT": np.ascontiguousarray(cat_prop[sl].T).astype(f16),
            "w_des": np.ascontiguousarray(
                np.asarray(W_des, f16).reshape(6, 128, 32)
                .transpose(1, 0, 2).reshape(128, 192)),
            "w_tweet": np.ascontiguousarray(
                np.asarray(W_tweet, f16).reshape(6, 128, 32)
                .transpose(1, 0, 2).reshape(128, 192)),
            "w_num": np.asarray(W_num, f16), "w_cat": np.asarray(W_cat, f16),
            "w_in": np.asarray(W_in, f16), "w_g1": np.asarray(W_g1, f16),
            "w_g2": np.asarray(W_g2, f16), "w_o1": np.asarray(W_o1, f16),
            "w_o2": np.asarray(W_o2, f16),
            "b_cat4": cat4_bias[:, None],
            "b_in": np.asarray(b_in, np.float32)[:, None],
            "c2": c2, "c3": c3,
            "b_o2r": b_o2r,
            "dinv_rep": np.tile(dv.astype(f16)[None, :], (128, 1)),
            "dinv_nm": dnm, "dinv2_nm": d2nm,
            "invd1p": (1.0 / dv).astype(f16)[None, :],
            "idx16": idx16[c],
            "dld": dld[c],
            "iota128": iota128,
            "sfix": sfix,
            "ident": np.eye(128, dtype=f16),
        }
        in_maps.append(m)

    global _last
    _last = (nc, in_maps)
    res = run_bass_kernel_spmd(nc, in_maps, core_ids=list(range(NCORES)))
    out = np.concatenate([res.results[c]["out"] for c in range(NCORES)],
                         axis=0)
    return out.astype(np.float32)


def prepare_run(**inputs):
    """Build (or fetch cached) program + per-core input maps, for benchmarking."""
    global _last
    kernel(**inputs)
    return _last


# revision 60
# speedup vs baseline: 1.2542x; 1.1963x over previous
"""BotGCN Trainium2 kernel: 8-core SPMD Bass/Tile implementation (V3).

Model: 4 input MLPs (768/768/6/11 -> 32 each) -> concat(128) -> Linear(128) ->
GCNConv -> GCNConv -> Linear(128) -> Linear(2), LeakyReLU(0.01) activations,
symmetric-normalized graph conv with self-loops on a 100K-node/3.2M-edge graph.

Sharding: nodes split contiguously across 8 cores (12500 each); edges assigned
to the core owning their dst.

Per conv: y = dinv*(W^T x) is computed node-major per 128-node tile (PE matmul
with the feature-major resident as lhsT), AllGathered into a replicated
[8*12501, 128] f16 table (one zero pad row per shard). The scatter runs as 4
sequential passes over 25002-row table banks (dma_gather indices are int16).
Within a pass, every dst owns exactly 8 slots per bank (self edge lives in the
dst's own bank; padding slots read the bank's zero row), so a 128-slot chunk
covers 16 dsts and multiplies with a CONSTANT one-hot [128,16]. Overflow edges
(>8 per dst/bank) go through a few dynamic one-hot chunks built on DVE.
Gathers run 64 chunks per SWDGE dma_gather instruction (994ns overhead
amortized); indices stream from DRAM. PSUM accumulates per 512-dst bank with
a PE zeroing matmul; pass 0 drains via scalar-engine copy, passes 1-3 via DVE
adds into the f16 feature-major z' resident.

Dst-side dinv and conv biases are deferred algebraically: z = dinv*z' + b is
never materialized; the next matmul uses dinv^2 scales plus a rank-1
((1/dinv) x W^T b) correction accumulated into PSUM.
"""
import sys
sys.path.insert(0, "/opt/trn_rl_repo")

import numpy as np

import concourse.bacc as bacc
import concourse.bass as bass
import concourse.mybir as mybir
import concourse.tile as tile
from concourse import library_config
from concourse.bass_utils import run_bass_kernel_spmd

F16 = mybir.dt.float16
F32 = mybir.dt.float32
I16 = mybir.dt.int16

NCORES = 8
N = 100000
E = 3200000
D = 128
SLOPE = 0.01
NC = N // NCORES            # 12500 nodes per core
NST = (NC + 127) // 128     # 98 128-dst supertiles per core
NBANK = (NC + 511) // 512   # 25 PSUM groups of 512 dsts
TPS = NST + 1               # row-runs per partition in the permuted shard
NCP1 = 128 * TPS            # shard rows (12672): node i at (i%128)*TPS+i//128
GB = 8                      # fixed gather slots per (dst, table-bank)
NTB = 4                     # table banks (int16 index range)
BNK = 2 * NCP1              # rows per table bank (25344 <= 32767)
KCH = 32                    # chunks per dma_gather instruction
GBUFS = 6                   # gather buffers in flight
GI = 8                      # gather instructions per idx-tile DMA
IBUFS = 3                   # idx-tile buffers
NODE_CHUNK = 500            # free-dim chunk for feature-major matmuls
NCHUNKS = NC // NODE_CHUNK
SB = 8                      # dyn one-hot group size (chunks per DVE op)

_cache = {}
_last = None
_DEBUG = False
_ABLATE = None  # None | "noscatter" | "nocoll" | "nogather"

# fixed chunks (16 dsts each) per supertile
FCS = [(min(128, NC - st * 128) + 15) // 16 for st in range(NST)]


def _build_program(dyncpt):
    """Build the SPMD Bass program.

    dyncpt: [NTB][NST] overflow chunk count per (table bank, supertile),
    uniform across cores (max-padded).
    """
    ndyn = int(sum(sum(b) for b in dyncpt))
    ndmax = max(1, max(max(b) for b in dyncpt))
    nfix = int(sum(FCS)) * NTB
    nch = nfix + ndyn                     # total chunks per conv
    nslot = nch * 128

    nc = bacc.Bacc("TRN2", target_bir_lowering=False, debug=False,
                   num_devices=NCORES, num_swdge_queues=4)

    # des/tweet interleaved per node-chunk: [128, chunk, ktile, col] so each
    # chunk loads with one DMA of 6KB-contiguous per-partition descriptors
    desT = nc.dram_tensor("desT", [128, 6 * NC], F16, kind="ExternalInput")
    tweetT = nc.dram_tensor("tweetT", [128, 6 * NC], F16,
                            kind="ExternalInput")
    numT = nc.dram_tensor("numT", [6, NC], F16, kind="ExternalInput")
    catT = nc.dram_tensor("catT", [11, NC], F16, kind="ExternalInput")
    w_des = nc.dram_tensor("w_des", [128, 6 * 32], F16, kind="ExternalInput")
    w_tweet = nc.dram_tensor("w_tweet", [128, 6 * 32], F16, kind="ExternalInput")
    w_num = nc.dram_tensor("w_num", [6, 32], F16, kind="ExternalInput")
    w_cat = nc.dram_tensor("w_cat", [11, 32], F16, kind="ExternalInput")
    w_in = nc.dram_tensor("w_in", [128, 128], F16, kind="ExternalInput")
    w_g1 = nc.dram_tensor("w_g1", [128, 128], F16, kind="ExternalInput")
    w_g2 = nc.dram_tensor("w_g2", [128, 128], F16, kind="ExternalInput")
    w_o1 = nc.dram_tensor("w_o1", [128, 128], F16, kind="ExternalInput")
    w_o2 = nc.dram_tensor("w_o2", [128, 2], F16, kind="ExternalInput")
    b_cat4 = nc.dram_tensor("b_cat4", [128, 1], F32, kind="ExternalInput")
    b_in = nc.dram_tensor("b_in", [128, 1], F32, kind="ExternalInput")
    c2 = nc.dram_tensor("c2", [1, 128], F16, kind="ExternalInput")
    c3 = nc.dram_tensor("c3", [128, 1], F32, kind="ExternalInput")
    b_o2r = nc.dram_tensor("b_o2r", [128, 2], F32, kind="ExternalInput")
    dinv_rep = nc.dram_tensor("dinv_rep", [128, NC], F16, kind="ExternalInput")
    dinv_nm = nc.dram_tensor("dinv_nm", [128, NST], F32, kind="ExternalInput")
    dinv2_nm = nc.dram_tensor("dinv2_nm", [128, NST], F32,
                              kind="ExternalInput")
    invd1p = nc.dram_tensor("invd1p", [1, NC], F16, kind="ExternalInput")
    idx16 = nc.dram_tensor("idx16", [128, nslot // 16], I16,
                           kind="ExternalInput")
    dld = nc.dram_tensor("dld", [128, max(ndyn, 1)], F16,
                         kind="ExternalInput")
    iota128 = nc.dram_tensor("iota128", [128, 128], F16, kind="ExternalInput")
    sfix = nc.dram_tensor("sfix", [128, 16], F16, kind="ExternalInput")
    ident = nc.dram_tensor("ident", [128, 128], F16, kind="ExternalInput")

    out = nc.dram_tensor("out", [NC, 2], F32, kind="ExternalOutput")
    if _DEBUG:
        dbg_y = nc.dram_tensor("dbg_y", [128, NST * 128], F16,
                               kind="ExternalOutput")
        dbg_z1 = nc.dram_tensor("dbg_z1", [128, NC], F16,
                                kind="ExternalOutput")

    with tile.TileContext(nc) as tc:
        with (
            tc.tile_pool(name="const", bufs=1) as cpool,
            tc.tile_pool(name="resid", bufs=1) as rpool,
            tc.tile_pool(name="stream", bufs=3) as spool,
            tc.tile_pool(name="pbig", bufs=2) as bpool,
            tc.tile_pool(name="psum", bufs=4, space="PSUM") as ppool,
            tc.tile_pool(name="gather", bufs=GBUFS) as gpool,
            tc.tile_pool(name="gidx", bufs=IBUFS) as ipool,
            tc.tile_pool(name="sbuild", bufs=2) as sbpool,
            tc.tile_pool(name="zpsum", bufs=3, space="PSUM") as zpool,
            tc.tile_pool(name="dram", bufs=1, space="DRAM") as dpool,
        ):
            nc.gpsimd.load_library(library_config.mlp)

            def load_const(t, shape, dt, name):
                s = cpool.tile(shape, dt, name=name)
                nc.sync.dma_start(s[:], t[:])
                return s

            s_wdes = load_const(w_des, [128, 6 * 32], F16, "s_wdes")
            s_wtweet = load_const(w_tweet, [128, 6 * 32], F16, "s_wtweet")
            s_wnum = load_const(w_num, [6, 32], F16, "s_wnum")
            s_wcat = load_const(w_cat, [11, 32], F16, "s_wcat")
            s_win = load_const(w_in, [128, 128], F16, "s_win")
            s_wg1 = load_const(w_g1, [128, 128], F16, "s_wg1")
            s_wg2 = load_const(w_g2, [128, 128], F16, "s_wg2")
            s_wo1 = load_const(w_o1, [128, 128], F16, "s_wo1")
            s_wo2 = load_const(w_o2, [128, 2], F16, "s_wo2")
            s_bcat4 = load_const(b_cat4, [128, 1], F32, "s_bcat4")
            s_bin = load_const(b_in, [128, 1], F32, "s_bin")
            s_c2 = load_const(c2, [1, 128], F16, "s_c2")
            s_c3 = load_const(c3, [128, 1], F32, "s_c3")
            s_bo2r = load_const(b_o2r, [128, 2], F32, "s_bo2r")
            s_dnm = load_const(dinv_nm, [128, NST], F32, "s_dnm")
            s_d2nm = load_const(dinv2_nm, [128, NST], F32, "s_d2nm")
            s_invd = load_const(invd1p, [1, NC], F16, "s_invd")
            s_dld = load_const(dld, [128, max(ndyn, 1)], F16, "s_dld")
            s_iota = load_const(iota128, [128, 128], F16, "s_iota")
            s_sfix = load_const(sfix, [128, 16], F16, "s_sfix")
            s_ident = load_const(ident, [128, 128], F16, "s_ident")

            # big feature-major residents; disjoint lifetimes share slots
            x2T = rpool.tile([128, NC], F16, name="x2T", tag="bigA")
            ystage = rpool.tile([128, TPS * 128], F16, name="ystage",
                                tag="bigY")

            y_shards = [dpool.tile([NCP1, D], F16, name=f"y_shard{i}")
                        for i in range(2)]
            y_fulls = [dpool.tile([NCORES * NCP1, D], F16, name=f"y_full{i}",
                                  addr_space="Shared") for i in range(2)]

            lrelu = mybir.ActivationFunctionType.Lrelu
            copyf = mybir.ActivationFunctionType.Copy

            # zero pad block lives at t=NST of ystage (zeroed once); z512
            # zeroes PSUM banks via PE
            zt = cpool.tile([128, 128], F16, name="zrow")
            nc.vector.memset(zt[:], 0.0)
            z512 = cpool.tile([1, 512], F16, name="z512")
            nc.vector.memset(z512[:], 0.0)
            nc.vector.memset(ystage[:, NST * 128:], 0.0)
            # unwritten ystage rows (tail tile) must be zero, not garbage;
            # partition offset must be 32-aligned, rows 64..83 are rewritten
            # by the conv y-stage afterwards
            nc.vector.memset(ystage[64:, (NST - 1) * 128:NST * 128], 0.0)

            # ================= Phase 1: input MLPs =================
            for ci in range(NCHUNKS):
                sl = slice(ci * NODE_CHUNK, (ci + 1) * NODE_CHUNK)
                ps = ppool.tile([128, NODE_CHUNK], F32, tag="ps", name="p1ps")
                # one 6KB-descriptor DMA per chunk for each 768-dim input
                xdes = bpool.tile([128, 6 * NODE_CHUNK], F16, tag="pdes",
                                  name="pdes")
                nc.sync.dma_start(
                    xdes[:], desT[:, ci * 6 * NODE_CHUNK:
                                  (ci + 1) * 6 * NODE_CHUNK])
                xtw = bpool.tile([128, 6 * NODE_CHUNK], F16, tag="ptw",
                                 name="ptw")
                nc.scalar.dma_start(
                    xtw[:], tweetT[:, ci * 6 * NODE_CHUNK:
                                   (ci + 1) * 6 * NODE_CHUNK])
                for base, wT, big, xT, kdim in (
                    (0, s_wdes, xdes, None, 768),
                    (32, s_wtweet, xtw, None, 768),
                    (64, s_wnum, None, numT, 6),
                    (96, s_wcat, None, catT, 11),
                ):
                    nkt = (kdim + 127) // 128
                    for kt in range(nkt):
                        kw = min(128, kdim - kt * 128)
                        if big is not None:
                            wslice = wT[:, kt * 32:(kt + 1) * 32]
                            operand = big[:, kt * NODE_CHUNK:
                                          (kt + 1) * NODE_CHUNK]
                        else:
                            wslice = wT[:kw, :]
                            xt = spool.tile([128, NODE_CHUNK], F16,
                                            tag="p1x", name="p1x")
                            nc.sync.dma_start(xt[:kw, :], xT[:kw, sl])
                            operand = xt[:kw, :]
                        nc.tensor.matmul(
                            ps[base:base + 32, :],
                            wslice,
                            operand,
                            start=(kt == 0),
                            stop=(kt == nkt - 1),
                            tile_position=(0, base),
                        )
                x1t = spool.tile([128, NODE_CHUNK], F16, tag="x1t", name="x1t")
                nc.scalar.activation(x1t[:], ps[:], lrelu,
                                     bias=s_bcat4[:], alpha=SLOPE)
                ps2 = ppool.tile([128, NODE_CHUNK], F32, tag="ps", name="p2ps")
                nc.tensor.matmul(ps2[:], s_win[:], x1t[:], start=True, stop=True)
                nc.scalar.activation(x2T[:, sl], ps2[:], lrelu,
                                     bias=s_bin[:], alpha=SLOPE)

            # ================= GCN convs =================
            qn = [0]

            def conv(src_fm, wg, scale_nm, rank1, y_shard, y_full, dst_fm):
                # ---- y-stage: node-major y tiles = scale * (W^T src) ----
                for t in range(NST):
                    w = min(128, NC - t * 128)
                    ps = ppool.tile([128, 128], F32, tag="ps", name="yps")
                    nc.tensor.matmul(ps[:w, :], src_fm[:, t * 128:t * 128 + w],
                                     wg[:], start=True, stop=not rank1)
                    if rank1:
                        nc.tensor.matmul(
                            ps[:w, :], s_invd[:, t * 128:t * 128 + w],
                            s_c2[:], start=False, stop=True,
                            skip_group_check=True)
                    nc.vector.tensor_scalar(
                        out=ystage[:w, t * 128:(t + 1) * 128],
                        in0=ps[:w, :], scalar1=scale_nm[:w, t:t + 1],
                        scalar2=None, op0=mybir.AluOpType.mult)

                # permuted shard: row (i%128)*TPS + i//128 <- one linear DMA
                nc.sync.dma_start(
                    y_shard[:].rearrange("(p t) f -> p t f", p=128),
                    ystage[:].rearrange("p (t f) -> p t f", f=128))

                if _ABLATE not in ("nocoll", "base"):
                    nc.gpsimd.collective_compute(
                        "AllGather", mybir.AluOpType.bypass,
                        replica_groups=[list(range(NCORES))],
                        ins=[y_shard.opt()], outs=[y_full.opt()],
                    )

                # ---- scatter: 4 banked passes over the y table ----
                ch = 0          # global chunk cursor (matches slot stream)
                dyn0 = 0        # dyn chunk cursor
                mt = None

                istate = [None, 0]  # current idx tile, base chunk

                def fetch(ch, bank, pch0, pnch):
                    """Gather next chunks of this pass (bank-local)."""
                    n = min(KCH, pnch - (ch - pch0))
                    if istate[0] is None or (ch - istate[1] + n) > GI * KCH:
                        ni = min(GI * KCH, nch - ch)
                        it = ipool.tile([128, GI * KCH * 8], I16, tag="idx",
                                        name="idx")
                        nc.sync.dma_start(it[:, :ni * 8],
                                          idx16[:, ch * 8: ch * 8 + ni * 8])
                        istate[0] = it
                        istate[1] = ch
                    it = istate[0]
                    i0 = (ch - istate[1]) * 8
                    t_ = gpool.tile([128, KCH * 128], F16, tag="mt",
                                    name="mt")
                    nc.gpsimd.dma_gather(
                        t_[:, :n * 128].rearrange("p (c f) -> p c f", f=128),
                        y_full[bank * BNK:(bank + 1) * BNK, :],
                        it[:, i0:i0 + n * 8], n * 128, n * 128, 128,
                        queue_num=qn[0] % 4, single_packet=False)
                    qn[0] += 1
                    return t_, ch + n

                if _ABLATE in ("noscatter", "base"):
                    for g in range(NBANK):
                        bw = min(512, NC - g * 512)
                        zps = zpool.tile([128, 512], F32, tag="zps",
                                         name="zps")
                        nc.tensor.matmul(zps[:], zt[0:1, :], z512[:],
                                         start=True, stop=True,
                                         skip_group_check=True)
                        nc.scalar.activation(
                            dst_fm[:, g * 512: g * 512 + bw],
                            zps[:, :bw], copyf)
                    return

                for bank in range(NTB):
                    pch0 = ch
                    pnch = sum(FCS) + int(sum(dyncpt[bank]))
                    nxt = ch
                    for g in range(NBANK):
                        bw = min(512, NC - g * 512)
                        zps = zpool.tile([128, 512], F32, tag="zps",
                                         name="zps")
                        nc.tensor.matmul(zps[:], zt[0:1, :], z512[:],
                                         start=True, stop=False,
                                         skip_group_check=True)
                        sts = list(range(g * 4, min(g * 4 + 4, NST)))
                        nchg = sum(FCS[st] + dyncpt[bank][st] for st in sts)
                        done = 0
                        if bank == 0:
                            # self-loop term: transpose node-major y tiles
                            # straight from SBUF into the accumulator
                            for st in sts:
                                ws = min(128, NC - st * 128)
                                nc.tensor.matmul(
                                    zps[:, (st % 4) * 128:
                                        (st % 4) * 128 + ws],
                                    ystage[:ws, st * 128:(st + 1) * 128],
                                    s_ident[:ws, :ws],
                                    start=False, stop=False,
                                    skip_group_check=True)
                        for st in sts:
                            w0 = (st % 4) * 128
                            for c in range(FCS[st]):
                                if ch == nxt:
                                    mt, nxt = fetch(ch, bank, pch0, pnch)
                                    mtb = ch
                                lhs = mt[:, (ch - mtb) * 128:
                                         (ch - mtb + 1) * 128]
                                nc.tensor.matmul(
                                    zps[:, w0 + 16 * c: w0 + 16 * c + 16],
                                    lhs, s_sfix[:],
                                    start=False, stop=(done == nchg - 1),
                                    skip_group_check=True)
                                ch += 1
                                done += 1
                            nd = dyncpt[bank][st]
                            if nd > 0:
                                stile = sbpool.tile([128, ndmax * 128], F16,
                                                    tag="stile", name="stile")
                                for g0 in range(0, nd, SB):
                                    gn = min(SB, nd - g0)
                                    dl = s_dld[:, dyn0 + g0: dyn0 + g0 + gn]
                                    nc.vector.tensor_tensor(
                                        out=stile[:, g0 * 128:(g0 + gn) * 128]
                                        .rearrange("p (c w) -> p c w", c=gn),
                                        in0=s_iota[:].unsqueeze(1)
                                        .to_broadcast([128, gn, 128]),
                                        in1=dl.unsqueeze(2)
                                        .to_broadcast([128, gn, 128]),
                                        op=mybir.AluOpType.is_equal)
                                for j in range(nd):
                                    if ch == nxt:
                                        mt, nxt = fetch(ch, bank, pch0, pnch)
                                        mtb = ch
                                    lhs = mt[:, (ch - mtb) * 128:
                                             (ch - mtb + 1) * 128]
                                    nc.tensor.matmul(
                                        zps[:, w0:w0 + 128],
                                        lhs, stile[:, j * 128:(j + 1) * 128],
                                        start=False,
                                        stop=(done == nchg - 1),
                                        skip_group_check=True)
                                    ch += 1
                                    done += 1
                                dyn0 += nd
                        # drain/accumulate this PSUM group
                        if bank == 0:
                            nc.scalar.activation(
                                dst_fm[:, g * 512: g * 512 + bw],
                                zps[:, :bw], copyf)
                        else:
                            nc.vector.tensor_tensor(
                                out=dst_fm[:, g * 512: g * 512 + bw],
                                in0=zps[:, :bw],
                                in1=dst_fm[:, g * 512: g * 512 + bw],
                                op=mybir.AluOpType.add)
                assert ch == nch and dyn0 == ndyn

            z1 = rpool.tile([128, NC], F16, name="z1", tag="bigB")
            conv(x2T, s_wg1, s_dnm, False, y_shards[0], y_fulls[0], z1)
            if _DEBUG:
                nc.sync.dma_start(dbg_y[:], ystage[:])
                nc.sync.dma_start(dbg_z1[:], z1[:])
            z2 = rpool.tile([128, NC], F16, name="z2", tag="bigA")
            conv(z1, s_wg2, s_d2nm, True, y_shards[1], y_fulls[1], z2)

            # ================= Output head =================
            # ystage is dead after conv2's AllGather: reuse its slot for the
            # replicated dinv table the head needs
            s_dinvrep = rpool.tile([128, NC], F16, name="s_dinvrep",
                                   tag="bigY")
            nc.sync.dma_start(s_dinvrep[:], dinv_rep[:])
            o1T = rpool.tile([128, NC], F16, name="o1T", tag="bigB")
            for ci in range(NCHUNKS):
                sl = slice(ci * NODE_CHUNK, (ci + 1) * NODE_CHUNK)
                ps = ppool.tile([128, NODE_CHUNK], F32, tag="ps", name="o1ps")
                nc.tensor.matmul(ps[:], s_wo1[:], z2[:, sl],
                                 start=True, stop=True)
                t1 = spool.tile([128, NODE_CHUNK], F16, tag="o1t", name="o1t")
                nc.vector.tensor_tensor(out=t1[:], in0=ps[:],
                                        in1=s_dinvrep[:, sl],
                                        op=mybir.AluOpType.mult)
                nc.scalar.activation(o1T[:, sl], t1[:], lrelu,
                                     bias=s_c3[:], alpha=SLOPE)

            ostage = rpool.tile([128, 2 * NST], F32, name="ostage",
                                tag="ostage")
            for t in range(NST):
                nlo = t * 128
                nhi = min(nlo + 128, NC)
                w = nhi - nlo
                ps = ppool.tile([128, 2], F32, tag="ps", name="o2ps")
                nc.tensor.matmul(ps[:w, :], o1T[:, nlo:nhi], s_wo2[:],
                                 start=True, stop=True)
                nc.vector.tensor_tensor(out=ostage[:w, 2 * t:2 * t + 2],
                                        in0=ps[:w, :], in1=s_bo2r[:w, :],
                                        op=mybir.AluOpType.add)
            # out[t*128 + p, c] = ostage[p, 2t + c]
            nfull = (NST - 1) * 128  # 12416 full-tile rows
            nc.sync.dma_start(
                out[:nfull, :].rearrange("(t p) c -> p t c", p=128),
                ostage[:, : 2 * (NST - 1)]
                .rearrange("p (t c) -> p t c", c=2))
            nc.sync.dma_start(out[nfull:, :],
                              ostage[: NC - nfull, 2 * (NST - 1):])

    nc.compile()
    return nc


def _prepare(edge_index):
    """Host-side graph prep: banked fixed-capacity slots + overflow chunks."""
    src = np.asarray(edge_index[0], dtype=np.int64)
    dst = np.asarray(edge_index[1], dtype=np.int64)
    deg = np.bincount(dst, minlength=N).astype(np.float64) + 1.0
    dinv = (1.0 / np.sqrt(deg)).astype(np.float32)

    order = np.argsort(dst, kind="stable")
    src_s, dst_s = src[order], dst[order]
    offs = np.searchsorted(dst_s, np.arange(0, N + 1, NC))

    cores = []
    ocnts = np.zeros((NCORES, NTB, NST), dtype=np.int64)
    for c in range(NCORES):
        s0, s1 = offs[c], offs[c + 1]
        gsrc = src_s[s0:s1]
        dl = (dst_s[s0:s1] - c * NC).astype(np.int64)
        si = gsrc % NC
        row = (gsrc // NC) * NCP1 + (si % 128) * TPS + si // 128
        bke = row // BNK
        loc = (row - bke * BNK).astype(np.int16)
        own = c // 2
        # rank within (dst, bank)
        o2 = np.lexsort((bke, dl))
        dl2, bk2, lc2 = dl[o2], bke[o2], loc[o2]
        gidkey = dl2 * NTB + bk2
        gstart = np.searchsorted(gidkey, np.arange(NC * NTB + 1))
        r2 = np.arange(dl2.size) - gstart[gidkey]
        fmask = r2 < GB
        # padding entries point at spread-out zero rows (t=NST runs)
        t_ = np.arange(NC * NTB * GB) % 256
        zspread = ((t_ // 128) * NCP1 + (t_ % 128) * TPS + NST
                   ).astype(np.int16)
        fixed = zspread.reshape(NC, NTB, GB).copy()
        fixed[dl2[fmask], bk2[fmask], r2[fmask]] = lc2[fmask]
        # overflow, ordered by (bank, dst)
        olc = lc2[~fmask]
        odl = dl2[~fmask]
        obk = bk2[~fmask]
        oo = np.lexsort((odl, obk))
        olc, odl, obk = olc[oo], odl[oo], obk[oo]
        ost = odl // 128
        for b in range(NTB):
            m = obk == b
            ocnts[c, b] = np.bincount(ost[m], minlength=NST)
        cores.append((fixed, olc, odl, obk))

    dyncpt = tuple(
        tuple(int(x) for x in (ocnts[:, b, :].max(axis=0) + 127) // 128)
        for b in range(NTB))
    ndyn = int(sum(sum(b) for b in dyncpt))
    nfix = int(sum(FCS)) * NTB
    nch = nfix + ndyn
    nslot = nch * 128

    idx16 = np.zeros((NCORES, 128, nslot // 16), dtype=np.int16)
    dld = np.full((NCORES, 128, max(ndyn, 1)), -1.0, dtype=np.float16)
    for c in range(NCORES):
        fixed, olc, odl, obk = cores[c]
        locs = np.zeros(nslot, dtype=np.int16)
        pos = 0
        dyn0 = 0
        for b in range(NTB):
            m = obk == b
            blc, bdl = olc[m], odl[m]
            bst = bdl // 128
            o_starts = np.searchsorted(bst, np.arange(NST + 1))
            for st in range(NST):
                nds = min(128, NC - st * 128)
                fc = FCS[st]
                t_ = np.arange(fc * 16 * GB) % 256
                blk = ((t_ // 128) * NCP1 + (t_ % 128) * TPS + NST
                       ).astype(np.int16).reshape(fc * 16, GB)
                blk[:nds] = fixed[st * 128: st * 128 + nds, b, :]
                locs[pos:pos + fc * 128] = blk.reshape(-1)
                pos += fc * 128
                nd = dyncpt[b][st]
                if nd:
                    a, e = o_starts[st], o_starts[st + 1]
                    cnt = e - a
                    # dyn padding: spread data rows; their one-hot row is 0
                    buf = ((np.arange(nd * 128) * 131) % NCP1
                           ).astype(np.int16)
                    buf[:cnt] = blc[a:e]
                    lbuf = np.full(nd * 128, -1.0, dtype=np.float16)
                    lbuf[:cnt] = (bdl[a:e] - st * 128).astype(np.float16)
                    locs[pos:pos + nd * 128] = buf
                    dld[c, :, dyn0:dyn0 + nd] = lbuf.reshape(nd, 128).T
                    pos += nd * 128
                    dyn0 += nd
        assert pos == nslot and dyn0 == ndyn
        # wrapped int16 layout: index j lives at [j%16, j//16]
        iw = locs.reshape(-1, 16).T
        idx16[c] = np.tile(iw, (8, 1))
    return dinv, dyncpt, idx16, dld


def kernel(des, tweet, num_prop, cat_prop, edge_index, edge_type,
           W_des, b_des, W_tweet, b_tweet, W_num, b_num, W_cat, b_cat,
           W_in, b_in, W_g1, b_g1, W_g2, b_g2, W_o1, b_o1, W_o2, b_o2):
    des = np.asarray(des, dtype=np.float32)
    tweet = np.asarray(tweet, dtype=np.float32)
    num_prop = np.asarray(num_prop, dtype=np.float32)
    cat_prop = np.asarray(cat_prop, dtype=np.float32)
    edge_index = np.asarray(edge_index)

    dinv, dyncpt, idx16, dld = _prepare(edge_index)

    key = ("prog", dyncpt, _DEBUG, _ABLATE, KCH, GBUFS, IBUFS)
    if key not in _cache:
        _cache[key] = _build_program(dyncpt)
    nc = _cache[key]

    f16 = np.float16
    cat4_bias = np.concatenate(
        [np.asarray(b) for b in (b_des, b_tweet, b_num, b_cat)]
    ).astype(np.float32)
    iota128 = np.tile(np.arange(128, dtype=np.float16)[None, :], (128, 1))
    sfix = np.zeros((128, 16), dtype=np.float16)
    sfix[np.arange(128), np.arange(128) // GB] = 1.0
    b_o2r = np.tile(np.asarray(b_o2, dtype=np.float32)[None, :], (128, 1))
    c2 = (np.asarray(b_g1, np.float64) @ np.asarray(W_g2, np.float64)
          ).astype(f16)[None, :]
    c3 = (np.asarray(b_g2, np.float64) @ np.asarray(W_o1, np.float64)
          + np.asarray(b_o1, np.float64)).astype(np.float32)[:, None]

    in_maps = []
    for c in range(NCORES):
        sl = slice(c * NC, (c + 1) * NC)
        dv = dinv[sl]
        dnm = np.zeros((128, NST), dtype=np.float32)
        dnm.T.flat[:NC] = dv
        d2nm = np.zeros((128, NST), dtype=np.float32)
        d2nm.T.flat[:NC] = dv * dv
        def inter(a):
            # [NC, 768] -> [128, chunk, ktile, col] flattened
            t = a.T.astype(f16).reshape(6, 128, NCHUNKS, NODE_CHUNK)
            return np.ascontiguousarray(
                t.transpose(1, 2, 0, 3).reshape(128, 6 * NC))
        m = {
            "desT": inter(des[sl]),
            "tweetT": inter(tweet[sl]),
            "numT": np.ascontiguousarray(num_prop[sl].T).astype(f16),
            "catT": np.ascontiguousarray(cat_prop[sl].T).astype(f16),
            "w_des": np.ascontiguousarray(
                np.asarray(W_des, f16).reshape(6, 128, 32)
                .transpose(1, 0, 2).reshape(128, 192)),
            "w_tweet": np.ascontiguousarray(
                np.asarray(W_tweet, f16).reshape(6, 128, 32)
                .transpose(1, 0, 2).reshape(128, 192)),
            "w_num": np.asarray(W_num, f16), "w_cat": np.asarray(W_cat, f16),
            "w_in": np.asarray(W_in, f16), "w_g1": np.asarray(W_g1, f16),
            "w_g2": np.asarray(W_g2, f16), "w_o1": np.asarray(W_o1, f16),
            "w_o2": np.asarray(W_o2, f16),
            "b_cat4": cat4_bias[:, None],
            "b_in": np.asarray(b_in, np.float32)[:, None],
            "c2": c2, "c3": c3,
            "b_o2r": b_o2r,
            "dinv_rep": np.tile(dv.astype(f16)[None, :], (128, 1)),
            "dinv_nm": dnm, "dinv2_nm": d2nm,
            "invd1p": (1.0 / dv).astype(f16)[None, :],
            "idx16": idx16[c],
            "dld": dld[c],
            "iota128": iota128,
            "sfix": sfix,
            "ident": np.eye(128, dtype=f16),
        }
        in_maps.append(m)

    global _last
    _last = (nc, in_maps)
    res = run_bass_kernel_spmd(nc, in_maps, core_ids=list(range(NCORES)))
    out = np.concatenate([res.results[c]["out"] for c in range(NCORES)],
                         axis=0)
    return out.astype(np.float32)


def prepare_run(**inputs):
    """Build (or fetch cached) program + per-core input maps, for benchmarking."""
    global _last
    kernel(**inputs)
    return _last
